# revision 7
# baseline (speedup 1.0000x reference)
"""BERT multi-head attention forward on 8 Trainium2 NeuronCores.

Sharding: tensor-parallel over heads (16 heads -> 2 per core) for the QKV
projection and attention; per-(batch, query-half) AllToAlls redistribute the
attention outputs token-wise so each core computes the output projection for
its own token slices (no AllReduce needed).

v3 (pipelined, exp split across ACT+DVE):
  - All matmuls bf16 (1 col/cycle).  Scores for the two heads are row-tiled
    (lhsT base partitions 0/64 -> concurrent 64-contraction matmuls).
  - exp tiles are split between the Scalar engine (table exp, exact) and the
    Vector engine (Schraudolph int16 bit-trick, +-3% per weight, washes out
    in softmax) so neither engine gates the attention inner loop.
  - Emission order pipelines phases: QKV(b1) runs after attention(b0) with
    its PSUM->SBUF copies on the idle Scalar engine; outproj(b0) fills the
    attention(b1) window; AllToAlls are split per (batch, query-half) into
    4 small collectives so only the last eighth of outproj sits in the tail.
  - sum-of-exp rides row 64 of the PV output (ones column in V); the
    normalization reciprocal is read straight out of PSUM row 64 by
    reciprocal_approx_fast and broadcast over 64 partitions via a tiny
    f32 ones matmul.

Per-core layouts:
  xT      [E=1024, T=4096] bf16  x transposed (embed on partitions)
  wqkvT   [1024, 384] bf16       this core's Wqkv rows (qA qB kA kB vA vB), transposed
  qkvT    [384, 4096] bf16 SBUF  j rows: q(128) k(128) v(128); each 128 = headA(64)+headB(64)
  vaug    [128, 132] bf16        per (b,kt): headA V(0:64)+ones(64), headB V(66:130)+ones(130)
  scp     [128, 1024] f32 PSUM   scores for one (b,qc,kt,h): 128 keys x 1024 queries
  ex      [128, 1024] bf16       exp'd scores
  oaug    [65, 1024] f32 PSUM    rows 0-63 unnormalized attn out (d x q), row 64 sumexp
  concatT [128, 4096] bf16       this core's 2 heads' channels x all tokens (normalized)
  A2A     per (b,qc): blocks of [128 ch, 128 tok] bf16
  outT    [1024, 512] f32        output projection result; col = b*256 + qc*128 + t
"""

import numpy as np
from concourse import bacc, tile, bass_utils, mybir

F32 = mybir.dt.float32
BF16 = mybir.dt.bfloat16
I16 = mybir.dt.int16
AF = mybir.ActivationFunctionType
ALU = mybir.AluOpType

B, S, E, H, D = 2, 2048, 1024, 16, 64
T = B * S                  # 4096 tokens
N_CORES = 8
HPC = H // N_CORES         # 2 heads per core
TC = 1024                  # t-chunk for QKV projection (bf16 moving max)
QC = 1024                  # query chunk in attention
KT_S = S // 128            # 16 key tiles per batch
TPB = T // B // N_CORES    # 256 tokens per core per batch
TPQ = TPB // 2             # 128 tokens per core per (batch, qc)  (A2A block)

ALPHA = 128.0 * 0.125 / np.log(2.0)   # fold into W_k: s' = ALPHA * (q.k)
ACT_SCALE = float(np.log(2.0) / 128.0)  # exp(ACT_SCALE * s') == exp(0.125 * q.k)
SCHRAUD_DELTA = -7.0                 # tuning offset for the bit-trick bias

# exp engine split: tile (b,qc,kt,h) goes to DVE iff its slot index mod
# SCHRAUD_MOD falls in SCHRAUD_SLOTS (else ACT).
SCHRAUD_MOD = 8
SCHRAUD_SLOTS = (1, 4, 6)

_CACHE = {}


def _build(k_rep=1):
    key = (k_rep, SCHRAUD_MOD, SCHRAUD_SLOTS)
    if key in _CACHE:
        return _CACHE[key]
    nc = bacc.Bacc("TRN2", target_bir_lowering=False, debug=False, num_devices=N_CORES)

    xT = nc.dram_tensor("xT", [E, T], BF16, kind="ExternalInput").ap()
    wqkvT = nc.dram_tensor("wqkvT", [E, 3 * 128], BF16, kind="ExternalInput").ap()
    bqkv_d = nc.dram_tensor("bqkv_sb", [128, 3], F32, kind="ExternalInput").ap()
    woutT = nc.dram_tensor("woutT", [E, E], BF16, kind="ExternalInput").ap()
    bout_d = nc.dram_tensor("bout_sb", [128, 8], F32, kind="ExternalInput").ap()
    abias_d = nc.dram_tensor("abias_sb", [128, B * KT_S], F32, kind="ExternalInput").ap()
    sbias_d = nc.dram_tensor("sbias_sb", [128, B * KT_S], F32, kind="ExternalInput").ap()
    ident_d = nc.dram_tensor("ident", [128, 128], BF16, kind="ExternalInput").ap()
    chain_d = nc.dram_tensor("chain", [1, 128], F32, kind="ExternalInput").ap()

    outT_d = nc.dram_tensor("outT", [E, 2 * TPB], F32, kind="ExternalOutput").ap()
    chout_d = nc.dram_tensor("chain_out", [1, 128], F32, kind="ExternalOutput").ap()

    with tile.TileContext(nc) as tc:
        with tc.tile_pool(name="sb", bufs=1) as sb, \
             tc.tile_pool(name="ps", bufs=1, space="PSUM") as ps, \
             tc.tile_pool(name="dram", bufs=1, space="DRAM") as dram:

            # chain passthrough (timing harness hook; negligible cost)
            ch_sb = sb.tile([1, 128], F32)
            nc.sync.dma_start(ch_sb[:], chain_d[:])
            nc.vector.tensor_copy(ch_sb[:], ch_sb[:])
            nc.sync.dma_start(chout_d[:], ch_sb[:])

            # ---- constants ----
            bqkv_sb = sb.tile([128, 3], F32)
            bout_sb = sb.tile([128, 8], F32)
            abias_sb = sb.tile([128, B * KT_S], F32)
            sbias_sb = sb.tile([128, B * KT_S], F32)
            ident_sb = sb.tile([128, 128], BF16)
            ones_sb = sb.tile([1, 64], BF16)
            nc.sync.dma_start(bqkv_sb[:], bqkv_d[:])
            nc.sync.dma_start(bout_sb[:], bout_d[:])
            nc.sync.dma_start(abias_sb[:], abias_d[:])
            nc.sync.dma_start(sbias_sb[:], sbias_d[:])
            nc.sync.dma_start(ident_sb[:], ident_d[:])
            nc.vector.memset(ones_sb[:], 1.0)

            # ---- weights ----
            wq_sb = [sb.tile([128, 3 * 128], BF16, name=f"wq_{e}")
                     for e in range(8)]
            for e in range(8):
                nc.sync.dma_start(wq_sb[e][:], wqkvT[e * 128:(e + 1) * 128, :])
            wo_sb = [sb.tile([128, E], BF16, name=f"wo_{e}") for e in range(8)]
            for e in range(8):
                nc.sync.dma_start(wo_sb[e][:], woutT[e * 128:(e + 1) * 128, :])

            for _rep in range(k_rep):
                qkvT = [sb.tile([128, T], BF16, name=f"qkvT_{j}") for j in range(3)]
                concatT = sb.tile([128, T], BF16)
                vaug = {}

                # ---- QKV projection for one 1024-token chunk, plus V
                # transposes for the 8 key-tiles it covers ----
                def qkv_chunk(i):
                    b, i2 = divmod(i, 2)
                    xt = [sb.tile([128, TC], BF16, name="xt", tag=f"xt{e}",
                                  bufs=2) for e in range(8)]
                    for e in range(8):
                        nc.sync.dma_start(
                            xt[e][:],
                            xT[e * 128:(e + 1) * 128, i * TC:(i + 1) * TC])
                    for j in range(3):
                        acc = ps.tile([128, TC], F32, name="acc", tag="scp", bufs=2)
                        for half in range(TC // 512):
                            hs_ = slice(half * 512, (half + 1) * 512)
                            for e in range(8):
                                nc.tensor.matmul(
                                    acc[:, hs_],
                                    wq_sb[e][:, j * 128:(j + 1) * 128],
                                    xt[e][:, hs_],
                                    start=(e == 0), stop=(e == 7))
                        with nc.allow_low_precision(reason="bf16 qkv"):
                            nc.vector.tensor_scalar(
                                out=qkvT[j][:, i * TC:(i + 1) * TC],
                                in0=acc[:],
                                scalar1=bqkv_sb[:, j:j + 1], scalar2=None,
                                op0=ALU.add)
                    # V natural layout via PE transpose ([128,128] covers both
                    # heads), + ones cols
                    for kt in range(i2 * 8, i2 * 8 + 8):
                        v = sb.tile([128, 132], BF16, name=f"vaug_{b}_{kt}")
                        tp = ps.tile([128, 128], BF16, name="tp", tag="scp", bufs=2)
                        nc.tensor.transpose(
                            tp[:],
                            qkvT[2][:, b * S + kt * 128: b * S + (kt + 1) * 128],
                            ident_sb[:])
                        with nc.allow_low_precision(reason="bf16 v"):
                            nc.vector.tensor_copy(v[:, 0:64], tp[:, 0:64])
                            nc.vector.tensor_copy(v[:, 66:130], tp[:, 64:128])
                        nc.vector.memset(v[:, 64:65], 1.0)
                        nc.vector.memset(v[:, 130:131], 1.0)
                        vaug[b, kt] = v

                # ---- attention for one (batch, query-chunk) ----
                def attention(b, qc):
                    q0 = b * S + qc * QC
                    oaug = {h: ps.tile([65, QC], F32, name=f"oaug{h}",
                                       tag=f"oaug{h}")
                            for h in range(HPC)}
                    for kt in range(KT_S):
                        k0 = b * S + kt * 128
                        exs = {}
                        for h in range(HPC):
                            scp = ps.tile([128, QC], F32, name="sc",
                                          tag="scp", bufs=2)
                            for half in range(QC // 512):
                                hs_ = slice(half * 512, (half + 1) * 512)
                                nc.tensor.matmul(
                                    scp[:, hs_],
                                    qkvT[1][64 * h:64 * h + 64, k0:k0 + 128],
                                    qkvT[0][64 * h:64 * h + 64,
                                            q0 + half * 512:q0 + (half + 1) * 512],
                                    start=True, stop=True)
                            ex = sb.tile([128, QC], I16, name="ex",
                                         tag="ex", bufs=3)
                            col = b * KT_S + kt
                            slot = ((qc * KT_S + kt) * HPC + h) % SCHRAUD_MOD
                            if slot in SCHRAUD_SLOTS:
                                with nc.allow_low_precision(reason="schraudolph"):
                                    nc.vector.tensor_scalar(
                                        out=ex[:], in0=scp[:],
                                        scalar1=sbias_sb[:, col:col + 1],
                                        scalar2=0.0, op0=ALU.add, op1=ALU.max)
                            else:
                                with nc.allow_low_precision(reason="bf16 attn w"):
                                    nc.scalar.activation(
                                        ex[:].bitcast(BF16), scp[:], AF.Exp,
                                        scale=ACT_SCALE,
                                        bias=abias_sb[:, col:col + 1])
                            exs[h] = ex
                        for h in range(HPC):
                            for half in range(QC // 512):
                                hs_ = slice(half * 512, (half + 1) * 512)
                                nc.tensor.matmul(
                                    oaug[h][:, hs_],
                                    vaug[b, kt][:, 66 * h:66 * h + 65],
                                    exs[h][:, hs_].bitcast(BF16),
                                    start=(kt == 0), stop=(kt == KT_S - 1))
                    # tail: normalize into concatT (sumexp rows side by side
                    # on partition 0 -- engine partition base must be 0/32/64)
                    sums = sb.tile([1, HPC * QC], F32, name="sums", tag="sums", bufs=2)
                    for h in range(HPC):
                        nc.vector.tensor_copy(sums[:, h * QC:(h + 1) * QC],
                                              oaug[h][64:65, :])
                    rec = sb.tile([1, HPC * QC], F32, name="rec", tag="rec", bufs=2)
                    nc.vector.reciprocal_approx_fast(rec[:], sums[:])
                    recr = sb.tile([1, HPC * QC], BF16, name="recr",
                                   tag="recr", bufs=2)
                    with nc.allow_low_precision(reason="bf16 recip"):
                        nc.vector.tensor_copy(recr[:], rec[:])
                    for h in range(HPC):
                        rep = ps.tile([64, QC], F32, name="rep", tag="scp", bufs=2)
                        for half in range(QC // 512):
                            hs_ = slice(half * 512, (half + 1) * 512)
                            nc.tensor.matmul(
                                rep[:, hs_], ones_sb[:],
                                recr[:, h * QC + half * 512:
                                     h * QC + (half + 1) * 512],
                                start=True, stop=True)
                        reps = sb.tile([64, QC], BF16, name="reps",
                                       tag="reps", bufs=2)
                        with nc.allow_low_precision(reason="bf16 recip"):
                            nc.vector.tensor_copy(reps[:], rep[:])
                        with nc.allow_low_precision(reason="bf16 concat"):
                            nc.vector.tensor_mul(
                                concatT[64 * h:64 * h + 64, q0:q0 + QC],
                                oaug[h][0:64, :], reps[:])

                # ---- per-(b,qc) AllToAll (blocks of [128 ch, 128 tok]) ----
                a2a_out = {}
                def a2a(b, qc):
                    a2a_in = dram.tile([N_CORES * 128, TPQ], BF16,
                                       name=f"a2a_in_{b}_{qc}")
                    a2a_o = dram.tile([N_CORES * 128, TPQ], BF16,
                                      name=f"a2a_out_{b}_{qc}")
                    t0 = b * S + qc * QC
                    for j in range(N_CORES):
                        nc.sync.dma_start(
                            a2a_in[j * 128:(j + 1) * 128, :],
                            concatT[:, t0 + j * TPQ: t0 + (j + 1) * TPQ])
                    nc.gpsimd.collective_compute(
                        "AllToAll", mybir.AluOpType.bypass,
                        replica_groups=[list(range(N_CORES))],
                        ins=[a2a_in.opt()], outs=[a2a_o.opt()])
                    a2a_out[b, qc] = a2a_o

                # ---- output projection for this core's 128 tokens of (b,qc) ----
                def outproj(b, qc):
                    cs = [sb.tile([128, TPQ], BF16, name="cs", tag=f"cs{kt}", bufs=2)
                          for kt in range(8)]
                    for kt in range(8):
                        nc.sync.dma_start(cs[kt][:],
                                          a2a_out[b, qc][kt * 128:(kt + 1) * 128, :])
                    c0 = b * TPB + qc * TPQ
                    for eo in range(8):
                        facc = ps.tile([128, TPQ], F32, name="facc", tag="scp", bufs=2)
                        for kt in range(8):
                            nc.tensor.matmul(facc[:], wo_sb[kt][:, eo * 128:(eo + 1) * 128],
                                             cs[kt][:], start=(kt == 0), stop=(kt == 7))
                        osb = sb.tile([128, TPQ], F32, name="osb", tag="osb", bufs=2)
                        nc.vector.tensor_scalar(
                            out=osb[:], in0=facc[:],
                            scalar1=bout_sb[:, eo:eo + 1], scalar2=None, op0=ALU.add)
                        nc.sync.dma_start(
                            outT_d[eo * 128:(eo + 1) * 128, c0:c0 + TPQ], osb[:])

                # ---- pipelined emission order ----
                qkv_chunk(0)
                qkv_chunk(1)
                attention(0, 0)
                a2a(0, 0)
                attention(0, 1)
                a2a(0, 1)
                qkv_chunk(2)
                qkv_chunk(3)
                attention(1, 0)
                a2a(1, 0)
                outproj(0, 0)
                outproj(0, 1)
                attention(1, 1)
                outproj(1, 0)
                a2a(1, 1)
                outproj(1, 1)

    nc.compile()
    _CACHE[key] = nc
    return nc


def _host_prep(x, mask, Wqkv, bqkv, Wout, bout):
    import ml_dtypes
    bf16 = ml_dtypes.bfloat16
    x = np.ascontiguousarray(np.asarray(x, np.float32))
    Wqkv = np.asarray(Wqkv, np.float32)
    bqkv = np.asarray(bqkv, np.float32)
    Wout = np.asarray(Wout, np.float32)
    bout = np.asarray(bout, np.float32)
    mask = np.asarray(mask)

    xT = np.ascontiguousarray(x.reshape(T, E).T.astype(bf16))          # [E, T]
    m = mask.reshape(B, S)
    ab = np.where(m == 0, np.float32(-30000.0), np.float32(0.0)).astype(np.float32)
    abias_sb = np.ascontiguousarray(ab.reshape(B, KT_S, 128).transpose(2, 0, 1)
                                    .reshape(128, B * KT_S))
    # Schraudolph bias: beta for live keys, very negative for masked keys
    beta = np.float32(127.0 * 128.0 + SCHRAUD_DELTA)
    sb_b = np.where(m == 0, np.float32(-1e7), beta).astype(np.float32)
    sbias_sb = np.ascontiguousarray(sb_b.reshape(B, KT_S, 128).transpose(2, 0, 1)
                                    .reshape(128, B * KT_S))
    woutT = np.ascontiguousarray(Wout.T.astype(bf16))                  # [e_in, e_out]
    bout_sb = np.ascontiguousarray(bout.reshape(8, 128).T)
    ident = np.eye(128, dtype=np.float32).astype(bf16)
    chain = np.zeros((1, 128), np.float32)

    in_maps = []
    for c in range(N_CORES):
        hs = [HPC * c + i for i in range(HPC)]
        rows = []
        for tix in range(3):  # q, k, v
            scale = ALPHA if tix == 1 else 1.0
            for h in hs:
                rows.append(Wqkv[tix * E + h * D: tix * E + (h + 1) * D] * scale)
        Wc = np.concatenate(rows, axis=0)                              # [384, 1024]
        wqkvT_c = np.ascontiguousarray(Wc.T.astype(bf16))              # [1024, 384]
        brows = []
        for tix in range(3):
            scale = ALPHA if tix == 1 else 1.0
            for h in hs:
                brows.append(bqkv[tix * E + h * D: tix * E + (h + 1) * D] * scale)
        bq_c = np.concatenate(brows).reshape(3, 128).T                 # [128, 3]
        in_maps.append({
            "xT": xT, "wqkvT": wqkvT_c,
            "bqkv_sb": np.ascontiguousarray(bq_c.astype(np.float32)),
            "woutT": woutT, "bout_sb": bout_sb, "abias_sb": abias_sb,
            "sbias_sb": sbias_sb, "ident": ident, "chain": chain,
        })
    return in_maps


def _assemble(results):
    out = np.empty((B, S, E), np.float32)
    for c in range(N_CORES):
        outT_c = results[c]["outT"]                                    # [E, 2*TPB]
        for b in range(B):
            for qc in range(2):
                out[b, qc * QC + c * TPQ:qc * QC + (c + 1) * TPQ, :] = \
                    outT_c[:, b * TPB + qc * TPQ:b * TPB + (qc + 1) * TPQ].T
    return out


def kernel(x, mask, Wqkv, bqkv, Wout, bout):
    nc = _build()
    in_maps = _host_prep(x, mask, Wqkv, bqkv, Wout, bout)
    res = bass_utils.run_bass_kernel_spmd(nc, in_maps, core_ids=list(range(N_CORES)))
    return _assemble(res.results)


# revision 19
# speedup vs baseline: 1.1499x; 1.1499x over previous
"""BERT multi-head attention forward on 8 Trainium2 NeuronCores.

Sharding: tensor-parallel over heads (16 heads -> 2 per core) for the QKV
projection and attention; per-(batch, query-half) AllToAlls redistribute the
attention outputs token-wise so each core computes the output projection for
its own token slices (no AllReduce needed).

v3 (pipelined, exp split across ACT+DVE):
  - All matmuls bf16 (1 col/cycle).  Scores for the two heads are row-tiled
    (lhsT base partitions 0/64 -> concurrent 64-contraction matmuls).
  - exp tiles are split between the Scalar engine (table exp, exact) and the
    Vector engine (Schraudolph int16 bit-trick, +-3% per weight, washes out
    in softmax) so neither engine gates the attention inner loop.
  - Emission order pipelines phases: QKV(b1) runs after attention(b0) with
    its PSUM->SBUF copies on the idle Scalar engine; outproj(b0) fills the
    attention(b1) window; AllToAlls are split per (batch, query-half) into
    4 small collectives so only the last eighth of outproj sits in the tail.
  - sum-of-exp rides row 64 of the PV output (ones column in V); the
    normalization reciprocal is read straight out of PSUM row 64 by
    reciprocal_approx_fast and broadcast over 64 partitions via a tiny
    f32 ones matmul.

Per-core layouts:
  xT      [E=1024, T=4096] bf16  x transposed (embed on partitions)
  wqkvT   [1024, 384] bf16       this core's Wqkv rows (qA qB kA kB vA vB), transposed
  qkvT    [384, 4096] bf16 SBUF  j rows: q(128) k(128) v(128); each 128 = headA(64)+headB(64)
  vaug    [128, 132] bf16        per (b,kt): headA V(0:64)+ones(64), headB V(66:130)+ones(130)
  scp     [128, 1024] f32 PSUM   scores for one (b,qc,kt,h): 128 keys x 1024 queries
  ex      [128, 1024] bf16       exp'd scores
  oaug    [65, 1024] f32 PSUM    rows 0-63 unnormalized attn out (d x q), row 64 sumexp
  concatT [128, 4096] bf16       this core's 2 heads' channels x all tokens (normalized)
  A2A     per (b,qc): blocks of [128 ch, 128 tok] bf16
  outT    [1024, 512] f32        output projection result; col = b*256 + qc*128 + t
"""

import numpy as np
from concourse import bacc, tile, bass_utils, mybir

F32 = mybir.dt.float32
BF16 = mybir.dt.bfloat16
I16 = mybir.dt.int16
AF = mybir.ActivationFunctionType
ALU = mybir.AluOpType

B, S, E, H, D = 2, 2048, 1024, 16, 64
T = B * S                  # 4096 tokens
N_CORES = 8
HPC = H // N_CORES         # 2 heads per core
TC = 1024                  # t-chunk for QKV projection (bf16 moving max)
QC = 1024                  # query chunk in attention
KT_S = S // 128            # 16 key tiles per batch
TPB = T // B // N_CORES    # 256 tokens per core per batch
TPQ = TPB // 2             # 128 tokens per core per (batch, qc)  (A2A block)

ALPHA = 128.0 * 0.125 / np.log(2.0)   # fold into W_k: s' = ALPHA * (q.k)
ACT_SCALE = float(np.log(2.0) / 128.0)  # exp(ACT_SCALE * s') == exp(0.125 * q.k)
SCHRAUD_DELTA = -7.0                 # tuning offset for the bit-trick bias

# exp engine split: tile (b,qc,kt,h) goes to DVE iff its slot index mod
# SCHRAUD_MOD falls in SCHRAUD_SLOTS (else ACT).
SCHRAUD_MOD = 8
SCHRAUD_SLOTS = (1, 4, 6)

_CACHE = {}


def _build(k_rep=1):
    key = (k_rep, SCHRAUD_MOD, SCHRAUD_SLOTS)
    if key in _CACHE:
        return _CACHE[key]
    nc = bacc.Bacc("TRN2", target_bir_lowering=False, debug=False, num_devices=N_CORES)

    xT = nc.dram_tensor("xT", [E, T], BF16, kind="ExternalInput").ap()
    wqkvT = nc.dram_tensor("wqkvT", [E, 3 * 128], BF16, kind="ExternalInput").ap()
    bqkv_d = nc.dram_tensor("bqkv_sb", [128, 3], F32, kind="ExternalInput").ap()
    woutT = nc.dram_tensor("woutT", [E, E], BF16, kind="ExternalInput").ap()
    bout_d = nc.dram_tensor("bout_sb", [128, 8], F32, kind="ExternalInput").ap()
    abias_d = nc.dram_tensor("abias_sb", [128, B * KT_S], F32, kind="ExternalInput").ap()
    sbias_d = nc.dram_tensor("sbias_sb", [128, B * KT_S], F32, kind="ExternalInput").ap()
    ident_d = nc.dram_tensor("ident", [128, 128], BF16, kind="ExternalInput").ap()
    chain_d = nc.dram_tensor("chain", [1, 128], F32, kind="ExternalInput").ap()

    outT_d = nc.dram_tensor("outT", [E, 2 * TPB], F32, kind="ExternalOutput").ap()
    chout_d = nc.dram_tensor("chain_out", [1, 128], F32, kind="ExternalOutput").ap()

    with tile.TileContext(nc) as tc:
        with tc.tile_pool(name="sb", bufs=1) as sb, \
             tc.tile_pool(name="ps", bufs=1, space="PSUM") as ps, \
             tc.tile_pool(name="dram", bufs=1, space="DRAM") as dram:

            # chain passthrough (timing harness hook; negligible cost)
            ch_sb = sb.tile([1, 128], F32)
            nc.sync.dma_start(ch_sb[:], chain_d[:])
            nc.vector.tensor_copy(ch_sb[:], ch_sb[:])
            nc.sync.dma_start(chout_d[:], ch_sb[:])

            # ---- constants ----
            bqkv_sb = sb.tile([128, 3], F32)
            bout_sb = sb.tile([128, 8], F32)
            abias_sb = sb.tile([128, B * KT_S], F32)
            sbias_sb = sb.tile([128, B * KT_S], F32)
            ident_sb = sb.tile([128, 128], BF16)
            ones_sb = sb.tile([1, 64], BF16)
            nc.sync.dma_start(bqkv_sb[:], bqkv_d[:])
            nc.sync.dma_start(bout_sb[:], bout_d[:])
            nc.sync.dma_start(abias_sb[:], abias_d[:])
            nc.sync.dma_start(sbias_sb[:], sbias_d[:])
            nc.sync.dma_start(ident_sb[:], ident_d[:])
            nc.vector.memset(ones_sb[:], 1.0)

            # ---- weights ----
            wq_sb = [sb.tile([128, 3 * 128], BF16, name=f"wq_{e}")
                     for e in range(8)]
            for e in range(8):
                nc.sync.dma_start(wq_sb[e][:], wqkvT[e * 128:(e + 1) * 128, :])
            wo_sb = [sb.tile([128, E], BF16, name=f"wo_{e}") for e in range(8)]
            wout_loaded = []
            def load_wout():
                if wout_loaded:
                    return
                wout_loaded.append(True)
                for e in range(8):
                    nc.sync.dma_start(wo_sb[e][:], woutT[e * 128:(e + 1) * 128, :])

            for _rep in range(k_rep):
                qkvT = [sb.tile([128, T], BF16, name=f"qkvT_{j}") for j in range(3)]
                concatT = sb.tile([128, T], BF16)
                vaug = {}

                # ---- QKV projection for one 1024-token chunk, plus V
                # transposes for the 8 key-tiles it covers ----
                def qkv_chunk(i):
                    b, i2 = divmod(i, 2)
                    xt = [sb.tile([128, TC], BF16, name="xt", tag=f"xt{e}",
                                  bufs=2) for e in range(8)]
                    for e in range(8):
                        nc.sync.dma_start(
                            xt[e][:],
                            xT[e * 128:(e + 1) * 128, i * TC:(i + 1) * TC])
                    for j in range(3):
                        acc = ps.tile([128, TC], F32, name="acc", tag="scp", bufs=2)
                        for half in range(TC // 512):
                            hs_ = slice(half * 512, (half + 1) * 512)
                            for e in range(8):
                                nc.tensor.matmul(
                                    acc[:, hs_],
                                    wq_sb[e][:, j * 128:(j + 1) * 128],
                                    xt[e][:, hs_],
                                    start=(e == 0), stop=(e == 7))
                        with nc.allow_low_precision(reason="bf16 qkv"):
                            nc.vector.tensor_scalar(
                                out=qkvT[j][:, i * TC:(i + 1) * TC],
                                in0=acc[:],
                                scalar1=bqkv_sb[:, j:j + 1], scalar2=None,
                                op0=ALU.add)
                    # V natural layout via PE transpose ([128,128] covers both
                    # heads), + ones cols
                    for kt in range(i2 * 8, i2 * 8 + 8):
                        v = sb.tile([128, 132], BF16, name=f"vaug_{b}_{kt}")
                        tp = ps.tile([128, 128], BF16, name="tp", tag="scp", bufs=2)
                        nc.tensor.transpose(
                            tp[:],
                            qkvT[2][:, b * S + kt * 128: b * S + (kt + 1) * 128],
                            ident_sb[:])
                        with nc.allow_low_precision(reason="bf16 v"):
                            nc.vector.tensor_copy(v[:, 0:64], tp[:, 0:64])
                            nc.vector.tensor_copy(v[:, 66:130], tp[:, 64:128])
                        nc.vector.memset(v[:, 64:65], 1.0)
                        nc.vector.memset(v[:, 130:131], 1.0)
                        vaug[b, kt] = v

                # ---- attention for one (batch, query-chunk) ----
                oaug_live = {}
                tails = {}

                def attention(b, qc, kts=range(KT_S), tail=True):
                    q0 = b * S + qc * QC
                    if (b, qc) in oaug_live:
                        oaug = oaug_live[b, qc]
                    else:
                        oaug = {h: ps.tile([65, QC], F32, name=f"oaug{h}",
                                           tag=f"oaug{h}")
                                for h in range(HPC)}
                        oaug_live[b, qc] = oaug
                    for kt in kts:
                        k0 = b * S + kt * 128
                        exs = {}
                        for h in range(HPC):
                            scp = ps.tile([128, QC], F32, name="sc",
                                          tag="scp", bufs=2)
                            for half in range(QC // 512):
                                hs_ = slice(half * 512, (half + 1) * 512)
                                nc.tensor.matmul(
                                    scp[:, hs_],
                                    qkvT[1][64 * h:64 * h + 64, k0:k0 + 128],
                                    qkvT[0][64 * h:64 * h + 64,
                                            q0 + half * 512:q0 + (half + 1) * 512],
                                    start=True, stop=True)
                            ex = sb.tile([128, QC], I16, name="ex",
                                         tag="ex", bufs=3)
                            col = b * KT_S + kt
                            slot = ((qc * KT_S + kt) * HPC + h) % SCHRAUD_MOD
                            if slot in SCHRAUD_SLOTS:
                                with nc.allow_low_precision(reason="schraudolph"):
                                    nc.vector.tensor_scalar(
                                        out=ex[:], in0=scp[:],
                                        scalar1=sbias_sb[:, col:col + 1],
                                        scalar2=0.0, op0=ALU.add, op1=ALU.max)
                            else:
                                with nc.allow_low_precision(reason="bf16 attn w"):
                                    nc.scalar.activation(
                                        ex[:].bitcast(BF16), scp[:], AF.Exp,
                                        scale=ACT_SCALE,
                                        bias=abias_sb[:, col:col + 1])
                            exs[h] = ex
                        for h in range(HPC):
                            for half in range(QC // 512):
                                hs_ = slice(half * 512, (half + 1) * 512)
                                nc.tensor.matmul(
                                    oaug[h][:, hs_],
                                    vaug[b, kt][:, 66 * h:66 * h + 65],
                                    exs[h][:, hs_].bitcast(BF16),
                                    start=(kt == 0), stop=(kt == KT_S - 1))
                    if not tail:
                        return
                    # extract: pull the unnormalized sums (bf16) and sumexp
                    # rows out of PSUM right away so the oaug banks free up
                    # for the next query chunk; the normalize chain is
                    # emitted later (norm_tail) so it never head-of-line
                    # blocks the PE queue.
                    osum = {}
                    sums = sb.tile([1, HPC * QC], F32, name="sums", tag="sums", bufs=2)
                    for h in range(HPC):
                        o = sb.tile([64, QC], BF16, name="osum",
                                    tag=f"osum{h}", bufs=2)
                        with nc.allow_low_precision(reason="bf16 attn out"):
                            if h == 0:
                                nc.vector.tensor_copy(o[:], oaug[h][0:64, :])
                                nc.scalar.copy(sums[:, h * QC:(h + 1) * QC],
                                               oaug[h][64:65, :])
                            else:
                                nc.scalar.copy(o[:], oaug[h][0:64, :])
                                nc.vector.tensor_copy(
                                    sums[:, h * QC:(h + 1) * QC],
                                    oaug[h][64:65, :])
                        osum[h] = o
                    del oaug_live[b, qc]
                    tails[b, qc] = (osum, sums)

                def norm_tail(b, qc):
                    q0 = b * S + qc * QC
                    osum, sums = tails.pop((b, qc))
                    rec = sb.tile([1, HPC * QC], F32, name="rec", tag="rec", bufs=2)
                    recr = sb.tile([1, HPC * QC], BF16, name="recr",
                                   tag="recr", bufs=2)
                    for h in range(HPC):
                        hs = slice(h * QC, (h + 1) * QC)
                        nc.vector.reciprocal_approx_fast(rec[:, hs], sums[:, hs])
                        with nc.allow_low_precision(reason="bf16 recip"):
                            nc.scalar.copy(recr[:, hs], rec[:, hs])
                    for h in range(HPC):
                        rep = ps.tile([64, QC], F32, name="rep", tag="scp", bufs=2)
                        for half in range(QC // 512):
                            hs_ = slice(half * 512, (half + 1) * 512)
                            nc.tensor.matmul(
                                rep[:, hs_], ones_sb[:],
                                recr[:, h * QC + half * 512:
                                     h * QC + (half + 1) * 512],
                                start=True, stop=True)
                        reps = sb.tile([64, QC], BF16, name="reps",
                                       tag="reps", bufs=2)
                        with nc.allow_low_precision(reason="bf16 recip"):
                            nc.scalar.copy(reps[:], rep[:])
                        with nc.allow_low_precision(reason="bf16 concat"):
                            nc.vector.tensor_mul(
                                concatT[64 * h:64 * h + 64, q0:q0 + QC],
                                osum[h][:], reps[:])

                # ---- AllToAll: per-batch for b0, per-(b,qc) for b1 so only
                # a quarter-size collective sits in the tail ----
                a2a_out = {}
                def a2a(b, qc=None):
                    n = TPB if qc is None else TPQ
                    t0 = b * S + (0 if qc is None else qc * QC)
                    sfx = f"{b}" if qc is None else f"{b}_{qc}"
                    a2a_in = dram.tile([N_CORES * 128, n], BF16,
                                       name=f"a2a_in_{sfx}")
                    a2a_o = dram.tile([N_CORES * 128, n], BF16,
                                      name=f"a2a_out_{sfx}")
                    for j in range(N_CORES):
                        nc.sync.dma_start(
                            a2a_in[j * 128:(j + 1) * 128, :],
                            concatT[:, t0 + j * n: t0 + (j + 1) * n])
                    nc.gpsimd.collective_compute(
                        "AllToAll", mybir.AluOpType.bypass,
                        replica_groups=[list(range(N_CORES))],
                        ins=[a2a_in.opt()], outs=[a2a_o.opt()])
                    a2a_out[b, qc] = a2a_o

                # ---- output projection for this core's tokens ----
                cs_live = {}
                def outproj(b, qc=None, eos=None):
                    n = TPB if qc is None else TPQ
                    c0 = b * TPB + (0 if qc is None else qc * TPQ)
                    if (b, qc) in cs_live:
                        cs = cs_live[b, qc]
                    else:
                        cs = [sb.tile([128, n], BF16, name="cs", tag=f"cs{kt}",
                                      bufs=2) for kt in range(8)]
                        for kt in range(8):
                            nc.sync.dma_start(
                                cs[kt][:],
                                a2a_out[b, qc][kt * 128:(kt + 1) * 128, :])
                        cs_live[b, qc] = cs
                    for eo in (eos if eos is not None else range(8)):
                        facc = ps.tile([128, n], F32, name="facc", tag="scp", bufs=2)
                        for kt in range(8):
                            nc.tensor.matmul(facc[:], wo_sb[kt][:, eo * 128:(eo + 1) * 128],
                                             cs[kt][:], start=(kt == 0), stop=(kt == 7))
                        osb = sb.tile([128, n], F32, name="osb", tag="osb", bufs=2)
                        nc.vector.tensor_scalar(
                            out=osb[:], in0=facc[:],
                            scalar1=bout_sb[:, eo:eo + 1], scalar2=None, op0=ALU.add)
                        nc.sync.dma_start(
                            outT_d[eo * 128:(eo + 1) * 128, c0:c0 + n], osb[:])

                # ---- pipelined emission order ----
                lo, hi = range(0, 8), range(8, KT_S)
                qkv_chunk(0)
                attention(0, 0, kts=lo, tail=False)
                qkv_chunk(1)
                attention(0, 0, kts=hi)
                attention(0, 1, kts=range(0, 5), tail=False)
                norm_tail(0, 0)
                attention(0, 1, kts=range(5, KT_S))
                qkv_chunk(2)
                attention(1, 0, kts=range(0, 5), tail=False)
                norm_tail(0, 1)
                a2a(0)
                load_wout()
                attention(1, 0, kts=range(5, 8), tail=False)
                qkv_chunk(3)
                attention(1, 0, kts=hi)
                attention(1, 1, kts=range(0, 5), tail=False)
                norm_tail(1, 0)
                a2a(1, 0)
                attention(1, 1, kts=range(5, KT_S))
                outproj(0, eos=range(0, 4))
                norm_tail(1, 1)
                outproj(0, eos=range(4, 8))
                a2a(1, 1)
                outproj(1, 0)
                outproj(1, 1)

    nc.compile()
    _CACHE[key] = nc
    return nc


def _host_prep(x, mask, Wqkv, bqkv, Wout, bout):
    import ml_dtypes
    bf16 = ml_dtypes.bfloat16
    x = np.ascontiguousarray(np.asarray(x, np.float32))
    Wqkv = np.asarray(Wqkv, np.float32)
    bqkv = np.asarray(bqkv, np.float32)
    Wout = np.asarray(Wout, np.float32)
    bout = np.asarray(bout, np.float32)
    mask = np.asarray(mask)

    xT = np.ascontiguousarray(x.reshape(T, E).T.astype(bf16))          # [E, T]
    m = mask.reshape(B, S)
    ab = np.where(m == 0, np.float32(-30000.0), np.float32(0.0)).astype(np.float32)
    abias_sb = np.ascontiguousarray(ab.reshape(B, KT_S, 128).transpose(2, 0, 1)
                                    .reshape(128, B * KT_S))
    # Schraudolph bias: beta for live keys, very negative for masked keys
    beta = np.float32(127.0 * 128.0 + SCHRAUD_DELTA)
    sb_b = np.where(m == 0, np.float32(-1e7), beta).astype(np.float32)
    sbias_sb = np.ascontiguousarray(sb_b.reshape(B, KT_S, 128).transpose(2, 0, 1)
                                    .reshape(128, B * KT_S))
    woutT = np.ascontiguousarray(Wout.T.astype(bf16))                  # [e_in, e_out]
    bout_sb = np.ascontiguousarray(bout.reshape(8, 128).T)
    ident = np.eye(128, dtype=np.float32).astype(bf16)
    chain = np.zeros((1, 128), np.float32)

    in_maps = []
    for c in range(N_CORES):
        hs = [HPC * c + i for i in range(HPC)]
        rows = []
        for tix in range(3):  # q, k, v
            scale = ALPHA if tix == 1 else 1.0
            for h in hs:
                rows.append(Wqkv[tix * E + h * D: tix * E + (h + 1) * D] * scale)
        Wc = np.concatenate(rows, axis=0)                              # [384, 1024]
        wqkvT_c = np.ascontiguousarray(Wc.T.astype(bf16))              # [1024, 384]
        brows = []
        for tix in range(3):
            scale = ALPHA if tix == 1 else 1.0
            for h in hs:
                brows.append(bqkv[tix * E + h * D: tix * E + (h + 1) * D] * scale)
        bq_c = np.concatenate(brows).reshape(3, 128).T                 # [128, 3]
        in_maps.append({
            "xT": xT, "wqkvT": wqkvT_c,
            "bqkv_sb": np.ascontiguousarray(bq_c.astype(np.float32)),
            "woutT": woutT, "bout_sb": bout_sb, "abias_sb": abias_sb,
            "sbias_sb": sbias_sb, "ident": ident, "chain": chain,
        })
    return in_maps


def _assemble(results):
    out = np.empty((B, S, E), np.float32)
    for c in range(N_CORES):
        outT_c = results[c]["outT"]                                    # [E, 2*TPB]
        # batch 0: one A2A over the whole batch (tokens c*256..)
        out[0, c * TPB:(c + 1) * TPB, :] = outT_c[:, 0:TPB].T
        # batch 1: per-qc A2As (tokens qc*1024 + c*128..)
        for qc in range(2):
            out[1, qc * QC + c * TPQ:qc * QC + (c + 1) * TPQ, :] = \
                outT_c[:, TPB + qc * TPQ:TPB + (qc + 1) * TPQ].T
    return out


def kernel(x, mask, Wqkv, bqkv, Wout, bout):
    nc = _build()
    in_maps = _host_prep(x, mask, Wqkv, bqkv, Wout, bout)
    res = bass_utils.run_bass_kernel_spmd(nc, in_maps, core_ids=list(range(N_CORES)))
    return _assemble(res.results)


# revision 23
# speedup vs baseline: 1.2392x; 1.0776x over previous
"""BERT multi-head attention forward on 8 Trainium2 NeuronCores.

Sharding: tensor-parallel over heads (16 heads -> 2 per core) for the QKV
projection and attention; per-(batch, query-half) AllToAlls redistribute the
attention outputs token-wise so each core computes the output projection for
its own token slices (no AllReduce needed).

v4 (pipelined, exp split across ACT+DVE; HW ~246us vs 256-266us baseline):
  - All matmuls bf16 (1 col/cycle).  Score matmuls for the two heads get
    tile_position (0,0)/(64,0) auto-derived from lhsT base partitions.
  - exp tiles are split between the Scalar engine (table exp, exact) and
    the Vector engine (Schraudolph int16 bit-trick via tensor_scalar
    add-bias + max + int16 convert, +-3% per weight, washes out in
    softmax) with SCHRAUD_SLOTS of every SCHRAUD_MOD tiles on DVE, so
    neither engine gates the attention inner loop.
  - Emission order pipelines phases at kt-range granularity: attention on
    a batch's first 8 key tiles starts right after that batch's first QKV
    token chunk; the normalize chain of each query chunk is deferred into
    the next chunk's kt loop (norm_tail) so it never head-of-line blocks
    the in-order PE queue (oaug PSUM is freed immediately by extracting
    osum/sums to SBUF); outproj(b0) fills the attention(1,1) window.
  - AllToAll: one per-batch collective for b0, per-(b,qc) collectives for
    b1 so only a quarter-size collective plus outproj(1,qc1) sit in the
    exposed tail; outproj(1,0) covers the last collective's flight time.
  - sum-of-exp rides row 64 of the PV output (ones column in V); recip on
    DVE from an SBUF copy (reciprocal_approx_fast from PSUM directly, and
    f32-moving rep matmuls, produced garbage on HW - keep bf16 ones/recr),
    broadcast over 64 partitions via a tiny ones matmul.

Per-core layouts:
  xT      [E=1024, T=4096] bf16  x transposed (embed on partitions)
  wqkvT   [1024, 384] bf16       this core's Wqkv rows (qA qB kA kB vA vB), transposed
  qkvT    [384, 4096] bf16 SBUF  j rows: q(128) k(128) v(128); each 128 = headA(64)+headB(64)
  vaug    [128, 132] bf16        per (b,kt): headA V(0:64)+ones(64), headB V(66:130)+ones(130)
  scp     [128, 1024] f32 PSUM   scores for one (b,qc,kt,h): 128 keys x 1024 queries
  ex      [128, 1024] bf16       exp'd scores
  oaug    [65, 1024] f32 PSUM    rows 0-63 unnormalized attn out (d x q), row 64 sumexp
  concatT [128, 4096] bf16       this core's 2 heads' channels x all tokens (normalized)
  A2A     per (b,qc): blocks of [128 ch, 128 tok] bf16
  outT    [1024, 512] f32        output projection result; col = b*256 + qc*128 + t
"""

import numpy as np
from concourse import bacc, tile, bass_utils, mybir

F32 = mybir.dt.float32
BF16 = mybir.dt.bfloat16
I16 = mybir.dt.int16
AF = mybir.ActivationFunctionType
ALU = mybir.AluOpType

B, S, E, H, D = 2, 2048, 1024, 16, 64
T = B * S                  # 4096 tokens
N_CORES = 8
HPC = H // N_CORES         # 2 heads per core
TC = 1024                  # t-chunk for QKV projection (bf16 moving max)
QC = 1024                  # query chunk in attention
KT_S = S // 128            # 16 key tiles per batch
TPB = T // B // N_CORES    # 256 tokens per core per batch
TPQ = TPB // 2             # 128 tokens per core per (batch, qc)  (A2A block)

ALPHA = 128.0 * 0.125 / np.log(2.0)   # fold into W_k: s' = ALPHA * (q.k)
ACT_SCALE = float(np.log(2.0) / 128.0)  # exp(ACT_SCALE * s') == exp(0.125 * q.k)
SCHRAUD_DELTA = -7.0                 # tuning offset for the bit-trick bias

# exp engine split: tile (b,qc,kt,h) goes to DVE iff its slot index mod
# SCHRAUD_MOD falls in SCHRAUD_SLOTS (else ACT).
SCHRAUD_MOD = 8
SCHRAUD_SLOTS = (1, 4, 6)

QK_FP8 = True          # fp8e4m3 DoubleRow for the q,k thirds of the QKV matmul
QUP = 16.0             # upscale W_q into fp8 normal range; undone in PSUM copy
F8 = mybir.dt.float8e4

_CACHE = {}


def _build(k_rep=1):
    key = (k_rep, SCHRAUD_MOD, SCHRAUD_SLOTS, QK_FP8)
    if key in _CACHE:
        return _CACHE[key]
    nc = bacc.Bacc("TRN2", target_bir_lowering=False, debug=False, num_devices=N_CORES)

    xT = nc.dram_tensor("xT", [E, T], BF16, kind="ExternalInput").ap()
    wqkvT = nc.dram_tensor("wqkvT", [E, 3 * 128], BF16, kind="ExternalInput").ap()
    if QK_FP8:
        xT8 = nc.dram_tensor("xT8", [E, T], F8, kind="ExternalInput").ap()
        # wqk8[p, sub*256 + m] = Wqk_c[m, e = sub*128 + p], m: q(0:128) k(128:256)
        wqk8_d = nc.dram_tensor("wqk8", [128, 8 * 256], F8,
                                kind="ExternalInput").ap()
    bqkv_d = nc.dram_tensor("bqkv_sb", [128, 3], F32, kind="ExternalInput").ap()
    woutT = nc.dram_tensor("woutT", [E, E], BF16, kind="ExternalInput").ap()
    bout_d = nc.dram_tensor("bout_sb", [128, 8], F32, kind="ExternalInput").ap()
    abias_d = nc.dram_tensor("abias_sb", [128, B * KT_S], F32, kind="ExternalInput").ap()
    sbias_d = nc.dram_tensor("sbias_sb", [128, B * KT_S], F32, kind="ExternalInput").ap()
    ident_d = nc.dram_tensor("ident", [128, 128], BF16, kind="ExternalInput").ap()
    chain_d = nc.dram_tensor("chain", [1, 128], F32, kind="ExternalInput").ap()

    outT_d = nc.dram_tensor("outT", [E, 2 * TPB], F32, kind="ExternalOutput").ap()
    chout_d = nc.dram_tensor("chain_out", [1, 128], F32, kind="ExternalOutput").ap()

    with tile.TileContext(nc) as tc:
        with tc.tile_pool(name="sb", bufs=1) as sb, \
             tc.tile_pool(name="ps", bufs=1, space="PSUM") as ps, \
             tc.tile_pool(name="dram", bufs=1, space="DRAM") as dram:

            # chain passthrough (timing harness hook; negligible cost)
            ch_sb = sb.tile([1, 128], F32)
            nc.sync.dma_start(ch_sb[:], chain_d[:])
            nc.vector.tensor_copy(ch_sb[:], ch_sb[:])
            nc.sync.dma_start(chout_d[:], ch_sb[:])

            # ---- constants ----
            bqkv_sb = sb.tile([128, 3], F32)
            bout_sb = sb.tile([128, 8], F32)
            abias_sb = sb.tile([128, B * KT_S], F32)
            sbias_sb = sb.tile([128, B * KT_S], F32)
            ident_sb = sb.tile([128, 128], BF16)
            ones_sb = sb.tile([1, 64], BF16)
            nc.sync.dma_start(bqkv_sb[:], bqkv_d[:])
            nc.sync.dma_start(bout_sb[:], bout_d[:])
            nc.sync.dma_start(abias_sb[:], abias_d[:])
            nc.sync.dma_start(sbias_sb[:], sbias_d[:])
            nc.sync.dma_start(ident_sb[:], ident_d[:])
            nc.vector.memset(ones_sb[:], 1.0)

            # ---- weights ----
            wq_sb = [sb.tile([128, 3 * 128], BF16, name=f"wq_{e}")
                     for e in range(8)]
            for e in range(8):
                nc.sync.dma_start(wq_sb[e][:], wqkvT[e * 128:(e + 1) * 128, :])
            if QK_FP8:
                wqk8_sb = sb.tile([128, 8 * 256], F8, name="wqk8")
                nc.sync.dma_start(wqk8_sb[:], wqk8_d[:])
                wqk8_3d = wqk8_sb[:].rearrange("p (sub m) -> p sub m", sub=8)
            wo_sb = [sb.tile([128, E], BF16, name=f"wo_{e}") for e in range(8)]
            wout_loaded = []
            def load_wout():
                if wout_loaded:
                    return
                wout_loaded.append(True)
                for e in range(8):
                    nc.sync.dma_start(wo_sb[e][:], woutT[e * 128:(e + 1) * 128, :])

            for _rep in range(k_rep):
                qkvT = [sb.tile([128, T], BF16, name=f"qkvT_{j}") for j in range(3)]
                concatT = sb.tile([128, T], BF16)
                vaug = {}

                # ---- QKV projection for one 1024-token chunk, plus V
                # transposes for the 8 key-tiles it covers ----
                def qkv_chunk(i):
                    b, i2 = divmod(i, 2)
                    xt = [sb.tile([128, TC], BF16, name="xt", tag=f"xt{e}",
                                  bufs=2) for e in range(8)]
                    for e in range(8):
                        nc.sync.dma_start(
                            xt[e][:],
                            xT[e * 128:(e + 1) * 128, i * TC:(i + 1) * TC])
                    if QK_FP8:
                        xt8 = sb.tile([128, 8, TC], F8, name="xt8", tag="xt8",
                                      bufs=2)
                        for e in range(8):
                            nc.sync.dma_start(
                                xt8[:, e, :],
                                xT8[e * 128:(e + 1) * 128, i * TC:(i + 1) * TC])
                    for j in range(3):
                        acc = ps.tile([128, TC], F32, name="acc", tag="scp", bufs=2)
                        for half in range(TC // 512):
                            hs_ = slice(half * 512, (half + 1) * 512)
                            if QK_FP8 and j < 2:
                                for kp in range(4):
                                    nc.tensor.matmul(
                                        acc[:, hs_],
                                        wqk8_3d[:, 2 * kp:2 * kp + 2,
                                                j * 128:(j + 1) * 128],
                                        xt8[:, 2 * kp:2 * kp + 2, hs_],
                                        start=(kp == 0), stop=(kp == 3),
                                        perf_mode=mybir.MatmulPerfMode.DoubleRow)
                            else:
                                for e in range(8):
                                    nc.tensor.matmul(
                                        acc[:, hs_],
                                        wq_sb[e][:, j * 128:(j + 1) * 128],
                                        xt[e][:, hs_],
                                        start=(e == 0), stop=(e == 7))
                        with nc.allow_low_precision(reason="bf16 qkv"):
                            if QK_FP8 and j == 0:
                                # undo the QUP weight upscale, then add bias
                                nc.vector.tensor_scalar(
                                    out=qkvT[j][:, i * TC:(i + 1) * TC],
                                    in0=acc[:],
                                    scalar1=float(1.0 / QUP),
                                    scalar2=bqkv_sb[:, j:j + 1],
                                    op0=ALU.mult, op1=ALU.add)
                            else:
                                nc.vector.tensor_scalar(
                                    out=qkvT[j][:, i * TC:(i + 1) * TC],
                                    in0=acc[:],
                                    scalar1=bqkv_sb[:, j:j + 1], scalar2=None,
                                    op0=ALU.add)
                    # V natural layout via PE transpose ([128,128] covers both
                    # heads), + ones cols
                    for kt in range(i2 * 8, i2 * 8 + 8):
                        v = sb.tile([128, 132], BF16, name=f"vaug_{b}_{kt}")
                        tp = ps.tile([128, 128], BF16, name="tp", tag="scp", bufs=2)
                        nc.tensor.transpose(
                            tp[:],
                            qkvT[2][:, b * S + kt * 128: b * S + (kt + 1) * 128],
                            ident_sb[:])
                        with nc.allow_low_precision(reason="bf16 v"):
                            nc.vector.tensor_copy(v[:, 0:64], tp[:, 0:64])
                            nc.vector.tensor_copy(v[:, 66:130], tp[:, 64:128])
                        nc.vector.memset(v[:, 64:65], 1.0)
                        nc.vector.memset(v[:, 130:131], 1.0)
                        vaug[b, kt] = v

                # ---- attention for one (batch, query-chunk) ----
                oaug_live = {}
                tails = {}

                def attention(b, qc, kts=range(KT_S), tail=True):
                    q0 = b * S + qc * QC
                    if (b, qc) in oaug_live:
                        oaug = oaug_live[b, qc]
                    else:
                        oaug = {h: ps.tile([65, QC], F32, name=f"oaug{h}",
                                           tag=f"oaug{h}")
                                for h in range(HPC)}
                        oaug_live[b, qc] = oaug
                    for kt in kts:
                        k0 = b * S + kt * 128
                        exs = {}
                        for h in range(HPC):
                            scp = ps.tile([128, QC], F32, name="sc",
                                          tag="scp", bufs=2)
                            for half in range(QC // 512):
                                hs_ = slice(half * 512, (half + 1) * 512)
                                nc.tensor.matmul(
                                    scp[:, hs_],
                                    qkvT[1][64 * h:64 * h + 64, k0:k0 + 128],
                                    qkvT[0][64 * h:64 * h + 64,
                                            q0 + half * 512:q0 + (half + 1) * 512],
                                    start=True, stop=True)
                            ex = sb.tile([128, QC], I16, name="ex",
                                         tag="ex", bufs=3)
                            col = b * KT_S + kt
                            slot = ((qc * KT_S + kt) * HPC + h) % SCHRAUD_MOD
                            if slot in SCHRAUD_SLOTS:
                                with nc.allow_low_precision(reason="schraudolph"):
                                    nc.vector.tensor_scalar(
                                        out=ex[:], in0=scp[:],
                                        scalar1=sbias_sb[:, col:col + 1],
                                        scalar2=0.0, op0=ALU.add, op1=ALU.max)
                            else:
                                with nc.allow_low_precision(reason="bf16 attn w"):
                                    nc.scalar.activation(
                                        ex[:].bitcast(BF16), scp[:], AF.Exp,
                                        scale=ACT_SCALE,
                                        bias=abias_sb[:, col:col + 1])
                            exs[h] = ex
                        for h in range(HPC):
                            for half in range(QC // 512):
                                hs_ = slice(half * 512, (half + 1) * 512)
                                nc.tensor.matmul(
                                    oaug[h][:, hs_],
                                    vaug[b, kt][:, 66 * h:66 * h + 65],
                                    exs[h][:, hs_].bitcast(BF16),
                                    start=(kt == 0), stop=(kt == KT_S - 1))
                    if not tail:
                        return
                    # extract: pull the unnormalized sums (bf16) and sumexp
                    # rows out of PSUM right away so the oaug banks free up
                    # for the next query chunk; the normalize chain is
                    # emitted later (norm_tail) so it never head-of-line
                    # blocks the PE queue.
                    osum = {}
                    sums = sb.tile([1, HPC * QC], F32, name="sums", tag="sums", bufs=2)
                    for h in range(HPC):
                        o = sb.tile([64, QC], BF16, name="osum",
                                    tag=f"osum{h}", bufs=2)
                        with nc.allow_low_precision(reason="bf16 attn out"):
                            if h == 0:
                                nc.vector.tensor_copy(o[:], oaug[h][0:64, :])
                                nc.scalar.copy(sums[:, h * QC:(h + 1) * QC],
                                               oaug[h][64:65, :])
                            else:
                                nc.scalar.copy(o[:], oaug[h][0:64, :])
                                nc.vector.tensor_copy(
                                    sums[:, h * QC:(h + 1) * QC],
                                    oaug[h][64:65, :])
                        osum[h] = o
                    del oaug_live[b, qc]
                    tails[b, qc] = (osum, sums)

                def norm_tail(b, qc):
                    q0 = b * S + qc * QC
                    osum, sums = tails.pop((b, qc))
                    rec = sb.tile([1, HPC * QC], F32, name="rec", tag="rec", bufs=2)
                    recr = sb.tile([1, HPC * QC], BF16, name="recr",
                                   tag="recr", bufs=2)
                    for h in range(HPC):
                        hs = slice(h * QC, (h + 1) * QC)
                        nc.vector.reciprocal_approx_fast(rec[:, hs], sums[:, hs])
                        with nc.allow_low_precision(reason="bf16 recip"):
                            nc.scalar.copy(recr[:, hs], rec[:, hs])
                    for h in range(HPC):
                        rep = ps.tile([64, QC], F32, name="rep", tag="scp", bufs=2)
                        for half in range(QC // 512):
                            hs_ = slice(half * 512, (half + 1) * 512)
                            nc.tensor.matmul(
                                rep[:, hs_], ones_sb[:],
                                recr[:, h * QC + half * 512:
                                     h * QC + (half + 1) * 512],
                                start=True, stop=True)
                        reps = sb.tile([64, QC], BF16, name="reps",
                                       tag="reps", bufs=2)
                        with nc.allow_low_precision(reason="bf16 recip"):
                            nc.scalar.copy(reps[:], rep[:])
                        with nc.allow_low_precision(reason="bf16 concat"):
                            nc.vector.tensor_mul(
                                concatT[64 * h:64 * h + 64, q0:q0 + QC],
                                osum[h][:], reps[:])

                # ---- AllToAll: per-batch for b0, per-(b,qc) for b1 so only
                # a quarter-size collective sits in the tail ----
                a2a_out = {}
                def a2a(b, qc=None):
                    n = TPB if qc is None else TPQ
                    t0 = b * S + (0 if qc is None else qc * QC)
                    sfx = f"{b}" if qc is None else f"{b}_{qc}"
                    a2a_in = dram.tile([N_CORES * 128, n], BF16,
                                       name=f"a2a_in_{sfx}")
                    a2a_o = dram.tile([N_CORES * 128, n], BF16,
                                      name=f"a2a_out_{sfx}")
                    for j in range(N_CORES):
                        nc.sync.dma_start(
                            a2a_in[j * 128:(j + 1) * 128, :],
                            concatT[:, t0 + j * n: t0 + (j + 1) * n])
                    nc.gpsimd.collective_compute(
                        "AllToAll", mybir.AluOpType.bypass,
                        replica_groups=[list(range(N_CORES))],
                        ins=[a2a_in.opt()], outs=[a2a_o.opt()])
                    a2a_out[b, qc] = a2a_o

                # ---- output projection for this core's tokens ----
                cs_live = {}
                def outproj(b, qc=None, eos=None):
                    n = TPB if qc is None else TPQ
                    c0 = b * TPB + (0 if qc is None else qc * TPQ)
                    if (b, qc) in cs_live:
                        cs = cs_live[b, qc]
                    else:
                        cs = [sb.tile([128, n], BF16, name="cs", tag=f"cs{kt}",
                                      bufs=2) for kt in range(8)]
                        for kt in range(8):
                            nc.sync.dma_start(
                                cs[kt][:],
                                a2a_out[b, qc][kt * 128:(kt + 1) * 128, :])
                        cs_live[b, qc] = cs
                    for eo in (eos if eos is not None else range(8)):
                        facc = ps.tile([128, n], F32, name="facc", tag="scp", bufs=2)
                        for kt in range(8):
                            nc.tensor.matmul(facc[:], wo_sb[kt][:, eo * 128:(eo + 1) * 128],
                                             cs[kt][:], start=(kt == 0), stop=(kt == 7))
                        osb = sb.tile([128, n], F32, name="osb", tag="osb", bufs=2)
                        nc.vector.tensor_scalar(
                            out=osb[:], in0=facc[:],
                            scalar1=bout_sb[:, eo:eo + 1], scalar2=None, op0=ALU.add)
                        nc.sync.dma_start(
                            outT_d[eo * 128:(eo + 1) * 128, c0:c0 + n], osb[:])

                # ---- pipelined emission order ----
                lo, hi = range(0, 8), range(8, KT_S)
                qkv_chunk(0)
                attention(0, 0, kts=lo, tail=False)
                qkv_chunk(1)
                attention(0, 0, kts=hi)
                attention(0, 1, kts=range(0, 5), tail=False)
                norm_tail(0, 0)
                attention(0, 1, kts=range(5, KT_S))
                qkv_chunk(2)
                attention(1, 0, kts=range(0, 5), tail=False)
                norm_tail(0, 1)
                a2a(0)
                load_wout()
                attention(1, 0, kts=range(5, 8), tail=False)
                qkv_chunk(3)
                attention(1, 0, kts=hi)
                attention(1, 1, kts=range(0, 5), tail=False)
                norm_tail(1, 0)
                a2a(1, 0)
                attention(1, 1, kts=range(5, KT_S))
                outproj(0, eos=range(0, 4))
                norm_tail(1, 1)
                outproj(0, eos=range(4, 8))
                a2a(1, 1)
                outproj(1, 0)
                outproj(1, 1)

    nc.compile()
    _CACHE[key] = nc
    return nc


def _host_prep(x, mask, Wqkv, bqkv, Wout, bout):
    import ml_dtypes
    bf16 = ml_dtypes.bfloat16
    x = np.ascontiguousarray(np.asarray(x, np.float32))
    Wqkv = np.asarray(Wqkv, np.float32)
    bqkv = np.asarray(bqkv, np.float32)
    Wout = np.asarray(Wout, np.float32)
    bout = np.asarray(bout, np.float32)
    mask = np.asarray(mask)

    xT = np.ascontiguousarray(x.reshape(T, E).T.astype(bf16))          # [E, T]
    m = mask.reshape(B, S)
    ab = np.where(m == 0, np.float32(-30000.0), np.float32(0.0)).astype(np.float32)
    abias_sb = np.ascontiguousarray(ab.reshape(B, KT_S, 128).transpose(2, 0, 1)
                                    .reshape(128, B * KT_S))
    # Schraudolph bias: beta for live keys, very negative for masked keys
    beta = np.float32(127.0 * 128.0 + SCHRAUD_DELTA)
    sb_b = np.where(m == 0, np.float32(-1e7), beta).astype(np.float32)
    sbias_sb = np.ascontiguousarray(sb_b.reshape(B, KT_S, 128).transpose(2, 0, 1)
                                    .reshape(128, B * KT_S))
    woutT = np.ascontiguousarray(Wout.T.astype(bf16))                  # [e_in, e_out]
    bout_sb = np.ascontiguousarray(bout.reshape(8, 128).T)
    ident = np.eye(128, dtype=np.float32).astype(bf16)
    chain = np.zeros((1, 128), np.float32)

    in_maps = []
    for c in range(N_CORES):
        hs = [HPC * c + i for i in range(HPC)]
        rows = []
        for tix in range(3):  # q, k, v
            scale = ALPHA if tix == 1 else 1.0
            for h in hs:
                rows.append(Wqkv[tix * E + h * D: tix * E + (h + 1) * D] * scale)
        Wc = np.concatenate(rows, axis=0)                              # [384, 1024]
        wqkvT_c = np.ascontiguousarray(Wc.T.astype(bf16))              # [1024, 384]
        brows = []
        for tix in range(3):
            scale = ALPHA if tix == 1 else 1.0
            for h in hs:
                brows.append(bqkv[tix * E + h * D: tix * E + (h + 1) * D] * scale)
        bq_c = np.concatenate(brows).reshape(3, 128).T                 # [128, 3]
        in_maps.append({
            "xT": xT, "wqkvT": wqkvT_c,
            "bqkv_sb": np.ascontiguousarray(bq_c.astype(np.float32)),
            "woutT": woutT, "bout_sb": bout_sb, "abias_sb": abias_sb,
            "sbias_sb": sbias_sb, "ident": ident, "chain": chain,
        })
    return in_maps


def _assemble(results):
    out = np.empty((B, S, E), np.float32)
    for c in range(N_CORES):
        outT_c = results[c]["outT"]                                    # [E, 2*TPB]
        # batch 0: one A2A over the whole batch (tokens c*256..)
        out[0, c * TPB:(c + 1) * TPB, :] = outT_c[:, 0:TPB].T
        # batch 1: per-qc A2As (tokens qc*1024 + c*128..)
        for qc in range(2):
            out[1, qc * QC + c * TPQ:qc * QC + (c + 1) * TPQ, :] = \
                outT_c[:, TPB + qc * TPQ:TPB + (qc + 1) * TPQ].T
    return out


def kernel(x, mask, Wqkv, bqkv, Wout, bout):
    nc = _build()
    in_maps = _host_prep(x, mask, Wqkv, bqkv, Wout, bout)
    res = bass_utils.run_bass_kernel_spmd(nc, in_maps, core_ids=list(range(N_CORES)))
    return _assemble(res.results)
